# revision 17
# baseline (speedup 1.0000x reference)
"""Causal multi-head attention (B=4, S=2048, D=2048, H=16) on 8 TRN2 NeuronCores.

Sharding: core c = 2*b + g handles batch b (of 4) and head-group g (of 2,
8 heads each).  Megatron-style: q/k/v projections are column-parallel over
the head dimension, the output projection is row-parallel; the host sums
the two partial outputs per batch and adds the bias.

All matmuls run as fp32r (TF32-like full-rate mode on the PE). Softmax
skips the max-subtraction (scores are ~N(0,1); exp cannot overflow) so the
whole attention needs no partition-dim reductions: scores are computed
transposed [sk, sq], the softmax denominator comes from a ones-vector
matmul, and normalization is deferred to after attn@v.
"""

import math

import numpy as np

B, S, D = 4, 2048, 2048
H_TOTAL, DH = 16, 128
G = 2               # tensor-parallel head groups
HG = H_TOTAL // G   # 8 heads per group
F = HG * DH         # 1024 features per group
N_CORES = 8

_CACHE = {}


def _build_nc(iters=1):
    import concourse.mybir as mybir
    from concourse import bacc
    from concourse.tile import TileContext
    from concourse.masks import make_upper_triangular

    FP32R = mybir.dt.float32r
    F32 = mybir.dt.float32
    AF = mybir.ActivationFunctionType
    MUL = mybir.AluOpType.mult

    DT = D // 128    # 16 contraction tiles
    ST = S // 128    # 16 seq tiles
    FT = F // 128    # 8 feature tiles (= heads per group)
    SB = S // 512    # 4 seq blocks
    FB = F // 256    # 4 feature quarter-blocks (v projection)

    nc = bacc.Bacc("TRN2", target_bir_lowering=False, debug=False)
    xT = nc.dram_tensor("xT", [D, S], FP32R, kind="ExternalInput")
    wq = nc.dram_tensor("wq", [D, F], FP32R, kind="ExternalInput")
    wk = nc.dram_tensor("wk", [D, F], FP32R, kind="ExternalInput")
    wv = nc.dram_tensor("wv", [D, F], FP32R, kind="ExternalInput")
    wo = nc.dram_tensor("wo", [F, D], FP32R, kind="ExternalInput")
    out = nc.dram_tensor("partial", [S, D], F32, kind="ExternalOutput")
    qT_s = nc.dram_tensor("qT_s", [F, S], FP32R)
    kT_s = nc.dram_tensor("kT_s", [F, S], FP32R)
    v_s = nc.dram_tensor("v_s", [S, F], FP32R)
    oT_s = nc.dram_tensor("oT_s", [F, S], FP32R)

    with TileContext(nc) as tc:
        with tc.tile_pool(name="const", bufs=1) as cp:
            # Pair masks for the two diagonal j-pairs of each 512-wide sq
            # block: MP0 = [tri|ones | z128|tri|ones256] handles (j=4b,4b+1),
            # MP1 = [z256|tri|ones128 | z384|tri] handles (j=4b+2,4b+3).
            # Half h of pair p masks a diagonal at column (2p+h)*128.
            # Plain fp32: only a DVE multiply consumes them.
            masks = [cp.tile([128, 1024], F32, name=f"mp_{p}") for p in range(2)]
            for p in range(2):
                for hh in range(2):
                    a = 2 * p + hh
                    c0 = hh * 512
                    if a:
                        nc.gpsimd.memset(masks[p][:, c0 : c0 + a * 128], 0.0)
                    make_upper_triangular(
                        nc, masks[p][:, c0 + a * 128 : c0 + (a + 1) * 128],
                        val=1.0, diag=True,
                    )
                    if a < 3:
                        nc.gpsimd.memset(
                            masks[p][:, c0 + (a + 1) * 128 : c0 + 512], 1.0
                        )
            o32 = cp.tile([128, 1], F32)
            nc.gpsimd.memset(o32[:], 1.0)
            ones = cp.tile([128, 1], FP32R)
            nc.vector.tensor_copy(ones[:], o32[:])

            for _ in range(iters):
                # ---- phase 1: q/k/v projections -------------------------
                with (
                    tc.tile_pool(name="ph1", bufs=1) as p1,
                    tc.tile_pool(name="ps1", bufs=1, space="PSUM") as ps1,
                ):
                    xt = p1.tile([128, DT, S], FP32R)  # x.T fully resident

                    def load_wqkf(f):
                        wqf = p1.tile([128, DT, 128], FP32R, tag="wqf", bufs=2)
                        wkf = p1.tile([128, DT, 128], FP32R, tag="wkf", bufs=2)
                        fs = slice(f * 128, (f + 1) * 128)
                        nc.sync.dma_start(
                            out=wqf[:], in_=wq[:, fs].rearrange("(t p) f -> p t f", p=128)
                        )
                        nc.sync.dma_start(
                            out=wkf[:], in_=wk[:, fs].rearrange("(t p) f -> p t f", p=128)
                        )
                        return wqf, wkf

                    # f=0 weights first, then the sb=0 column block of x.T,
                    # so the first matmul chain only waits on ~6MB of DMA
                    wqkf0 = load_wqkf(0)
                    for sb in range(SB):
                        for d in range(DT):
                            nc.sync.dma_start(
                                out=xt[:, d, sb * 512 : (sb + 1) * 512],
                                in_=xT[d * 128 : (d + 1) * 128, sb * 512 : (sb + 1) * 512],
                            )

                    for f in range(FT):
                        wqf, wkf = wqkf0 if f == 0 else load_wqkf(f)
                        fs = slice(f * 128, (f + 1) * 128)
                        for sb in range(SB):
                            ss = slice(sb * 512, (sb + 1) * 512)
                            for w_t, dst in ((wqf, qT_s), (wkf, kT_s)):
                                acc = ps1.tile([128, 512], F32, tag="ps_qk", bufs=2)
                                for d in range(DT):
                                    nc.tensor.matmul(
                                        acc[:],
                                        w_t[:, d, :],
                                        xt[:, d, ss],
                                        start=(d == 0),
                                        stop=(d == DT - 1),
                                    )
                                ev = p1.tile([128, 512], FP32R, tag="ev_qk", bufs=2)
                                nc.vector.tensor_copy(ev[:], acc[:])
                                nc.sync.dma_start(out=dst[fs, ss], in_=ev[:])

                    for fb in range(FB):
                        wvb = p1.tile([128, DT, 256], FP32R, tag="wvb", bufs=2)
                        fbs = slice(fb * 256, (fb + 1) * 256)
                        nc.sync.dma_start(
                            out=wvb[:], in_=wv[:, fbs].rearrange("(t p) f -> p t f", p=128)
                        )
                        for st in range(ST):
                            acc = ps1.tile([128, 256], F32, tag="ps_v", bufs=2)
                            for d in range(DT):
                                nc.tensor.matmul(
                                    acc[:],
                                    xt[:, d, st * 128 : (st + 1) * 128],
                                    wvb[:, d, :],
                                    start=(d == 0),
                                    stop=(d == DT - 1),
                                )
                            ev = p1.tile([128, 256], FP32R, tag="ev_v", bufs=2)
                            nc.vector.tensor_copy(ev[:], acc[:])
                            nc.sync.dma_start(
                                out=v_s[st * 128 : (st + 1) * 128, fbs], in_=ev[:]
                            )

                # ---- phases 2+3 share one SBUF pool scope ---------------
                with tc.tile_pool(name="ph23", bufs=1) as p2:
                    wof = p2.tile([128, FT, D], FP32R)

                    # ---- phase 2: causal attention per head -------------
                    with (
                        tc.tile_pool(name="ps2s", bufs=1, space="PSUM") as ps2s,
                        tc.tile_pool(name="ps2o", bufs=1, space="PSUM") as ps2o,
                    ):
                        # Software-pipelined by two j-pairs: pair p's av/l
                        # matmuls are emitted after pair p+2's score matmuls,
                        # so the PE never sits behind p's exp (ACT) or the
                        # diagonal mask multiply (DVE).
                        DEPTH = 2
                        pend = []  # (pt, vh, acc_o, acc_l, j0, jmax)
                        epilogue = None  # accumulators of a finished block

                        def flush_pending():
                            nonlocal epilogue
                            if not pend:
                                return
                            pt_, vh_, acc_o_, acc_l_, j0_, jmax_ = pend.pop(0)
                            for hh in range(2):
                                j = j0_ + hh
                                # columns below the causal diagonal are zero in
                                # pt — skip them when that keeps N >= 256
                                # (full fp32r rate)
                                a = j - (jmax_ - 3)
                                c0 = a * 128 if a in (1, 2) else 0
                                pslice = pt_[:, hh * 512 + c0 : (hh + 1) * 512]
                                nc.tensor.matmul(
                                    acc_o_[:, c0:512], vh_[:, j, :], pslice,
                                    start=(j == 0), stop=(j == jmax_),
                                )
                                nc.tensor.matmul(
                                    acc_l_[:, c0:512], ones[:], pslice,
                                    start=(j == 0), stop=(j == jmax_),
                                )
                            if j0_ + 1 == jmax_:  # block finished
                                epilogue = (acc_o_, acc_l_)

                        def flush_epilogue(hs_, bs_):
                            nonlocal epilogue
                            assert epilogue is not None
                            acc_o_, acc_l_ = epilogue
                            epilogue = None
                            linv = p2.tile([1, 512], F32, tag="linv", bufs=2)
                            nc.vector.reciprocal(linv[:], acc_l_[:])
                            linb = p2.tile([128, 512], F32, tag="linb", bufs=2)
                            nc.gpsimd.partition_broadcast(linb[:], linv[:])
                            otb = p2.tile([128, 512], FP32R, tag="otb", bufs=3)
                            nc.vector.tensor_tensor(
                                out=otb[:], in0=acc_o_[:], in1=linb[:], op=MUL
                            )
                            nc.sync.dma_start(out=oT_s[hs_, bs_], in_=otb[:])

                        blocks = []  # (h, b) epilogue coords in flight
                        for h in range(HG):
                            hs = slice(h * 128, (h + 1) * 128)
                            qth = p2.tile([128, S], FP32R, tag="qth", bufs=3)
                            kth = p2.tile([128, S], FP32R, tag="kth", bufs=3)
                            vh = p2.tile([128, ST, DH], FP32R, tag="vh", bufs=3)
                            nc.sync.dma_start(out=qth[:], in_=qT_s[hs, :])
                            nc.sync.dma_start(out=kth[:], in_=kT_s[hs, :])
                            nc.sync.dma_start(
                                out=vh[:],
                                in_=v_s[:, hs].rearrange("(t p) f -> p t f", p=128),
                            )
                            if h == 0:
                                # prefetch the output-projection weight behind
                                # the first head's loads
                                nc.sync.dma_start(
                                    out=wof[:],
                                    in_=wo.rearrange("(t p) f -> p t f", p=128),
                                )
                            for b in range(SB):
                                bs = slice(b * 512, (b + 1) * 512)
                                acc_o = ps2o.tile([128, 512], F32, tag="ps_o", bufs=2)
                                acc_l = ps2o.tile([1, 512], F32, tag="ps_l", bufs=2)
                                jmax = 4 * b + 3
                                for jp in range(2 * b + 2):
                                    j0 = 2 * jp
                                    sc = ps2s.tile([128, 1024], F32, tag="ps_s", bufs=2)
                                    for hh in range(2):
                                        j = j0 + hh
                                        # causal: columns sq < j*128 are dead;
                                        # narrow when N stays >= 256. The
                                        # skipped psum region holds stale
                                        # (bounded) scores; exp of it is
                                        # finite and the pair mask zeroes it.
                                        a = j - 4 * b
                                        c0 = a * 128 if a in (1, 2) else 0
                                        nc.tensor.matmul(
                                            sc[:, hh * 512 + c0 : (hh + 1) * 512],
                                            kth[:, j * 128 : (j + 1) * 128],
                                            qth[:, b * 512 + c0 : (b + 1) * 512],
                                            start=True,
                                            stop=True,
                                        )
                                    pt = p2.tile([128, 1024], FP32R, tag="pt", bufs=4)
                                    nc.scalar.activation(pt[:], sc[:], AF.Exp)
                                    if j0 >= 4 * b:  # diagonal pair
                                        nc.vector.tensor_tensor(
                                            out=pt[:],
                                            in0=pt[:],
                                            in1=masks[jp - 2 * b][:],
                                            op=MUL,
                                        )
                                    pend.append((pt, vh, acc_o, acc_l, j0, jmax))
                                    if len(pend) > DEPTH:
                                        flush_pending()
                                        if epilogue is not None:
                                            flush_epilogue(*blocks.pop(0))
                                blocks.append((hs, bs))
                        while pend:
                            flush_pending()
                            if epilogue is not None:
                                flush_epilogue(*blocks.pop(0))

                    # ---- phase 3: output projection ---------------------
                    with tc.tile_pool(name="ps3", bufs=1, space="PSUM") as ps3:
                        for st in range(ST):
                            sts = slice(st * 128, (st + 1) * 128)
                            ot = p2.tile([128, FT, 128], FP32R, tag="ot", bufs=2)
                            nc.sync.dma_start(
                                out=ot[:],
                                in_=oT_s[:, sts].rearrange("(t p) s -> p t s", p=128),
                            )
                            for ob in range(SB):
                                obs = slice(ob * 512, (ob + 1) * 512)
                                acc = ps3.tile([128, 512], F32, tag="ps_p", bufs=2)
                                for f in range(FT):
                                    nc.tensor.matmul(
                                        acc[:],
                                        ot[:, f, :],
                                        wof[:, f, obs],
                                        start=(f == 0),
                                        stop=(f == FT - 1),
                                    )
                                po = p2.tile([128, 512], F32, tag="po", bufs=4)
                                nc.vector.tensor_copy(po[:], acc[:])
                                nc.sync.dma_start(out=out[sts, obs], in_=po[:])

    nc.compile()
    return nc


def _get_nc(iters=1):
    key = ("nc", iters)
    if key not in _CACHE:
        _CACHE[key] = _build_nc(iters)
    return _CACHE[key]


def make_in_maps(x, Wq, Wk, Wv, Wo):
    scale = 1.0 / math.sqrt(DH)
    xTs = [np.ascontiguousarray(x[b].T) for b in range(B)]
    in_maps = []
    for c in range(N_CORES):
        b, g = divmod(c, G)
        gs = slice(g * F, (g + 1) * F)
        in_maps.append(
            {
                "xT": xTs[b],
                "wq": np.ascontiguousarray(Wq[gs, :].T) * np.float32(scale),
                "wk": np.ascontiguousarray(Wk[gs, :].T),
                "wv": np.ascontiguousarray(Wv[gs, :].T),
                "wo": np.ascontiguousarray(Wo[:, gs].T),
            }
        )
    return in_maps


def kernel(x, Wq, Wk, Wv, Wo, bo):
    from concourse.bass_utils import run_bass_kernel_spmd

    x = np.asarray(x, dtype=np.float32)
    Wq = np.asarray(Wq, dtype=np.float32)
    Wk = np.asarray(Wk, dtype=np.float32)
    Wv = np.asarray(Wv, dtype=np.float32)
    Wo = np.asarray(Wo, dtype=np.float32)
    bo = np.asarray(bo, dtype=np.float32)

    nc = _get_nc()
    in_maps = make_in_maps(x, Wq, Wk, Wv, Wo)
    res = run_bass_kernel_spmd(nc, in_maps, list(range(N_CORES)))
    out = np.empty((B, S, D), dtype=np.float32)
    for b in range(B):
        out[b] = res.results[2 * b]["partial"] + res.results[2 * b + 1]["partial"] + bo
    return out
